# revision 20
# baseline (speedup 1.0000x reference)
"""Causal attention (B=4, S=4096, D=64, fp32) on 8 Trainium2 NeuronCores.

Strategy (v2)
-------------
Sharding: 2 cores per batch element; the two cores of a batch split the KV
blocks by parity (even / odd 128-row blocks). Each core computes, for every
query position of its batch, the *unnormalized* attention numerator and the
softmax denominator contribution of its own KV half. The
host sums the two halves and divides.

v2 splits the exp work across TWO engines (the v1 bottleneck was the ACT
engine's exp stream at ~41us busy):
  - ACT (scalar) engine: exact exp (fp32 PSUM -> fp16 SBUF) for the
    boundary (diagonal) pair of every q tile, the near-diagonal full pairs,
    and tile 0 -- the places where softmax weights concentrate and
    approximation error would show in the output.
  - DVE (vector) engine: Schraudolph exp for far-from-diagonal pairs: ONE
    tensor_scalar instruction computes int16(round(a*(x/8) + b)) from the
    fp32 PSUM scores; the int16 bit pattern IS the fp16 encoding of
    ~exp(x/8) (+-3% sawtooth; far pairs carry little softmax weight and the
    per-row softmax normalization cancels the common mode, so the output
    error stays ~3e-3, vs the 2e-2 gate). fp8+DoubleRow PV was evaluated and
    rejected: e4m3 cannot span the score range without NaN/negative-bitcast
    tails, e5m2's 2-bit mantissa costs 1.4e-2 end-to-end.
  - Causal masking: only the [128,128] diagonal sub-block of each boundary
    block is actually triangular; fully-masked columns are excluded by the
    matmul/exp/PV column ranges. DVE multiplies the two diagonal sub-blocks
    by one shared lower-triangle mask (~260ns/tile).
Scores: S_T[kv, q] = K @ Q^T in fp16, row-tiled pairs via tile_position.
Scheduling: a global software pipeline with LAG=4 pairs -- the PE emits the
scores matmuls of pair i+4 before the exp+PV of pair i, with 6 single-bank
PSUM score tiles (one per kv block) and exp split into per-block halves, so
no engine's in-order queue ever drains (queue drains collapse the PE's HAM
clock gate to 1.2 GHz and convoy the whole pipeline -- measured 58us vs 50us
for the identical instruction set emitted pair-lockstep vs pipelined).
Host: transposes Q/K, packs per-core inputs (fp16 V|1 blocks + ones column
so PV's row 64 is the softmax denominator), combines/normalizes halves.
"""

import math
import numpy as np
from contextlib import ExitStack

import concourse.tile as tile
from concourse import bacc, mybir
from concourse.bass_utils import run_bass_kernel_spmd

B, S, D = 4, 4096, 64
NCORES = 8
BLK = 128            # kv block rows
QTW = 512            # q tile width
NQT = S // QTW       # 8 q tiles
PAR = S // BLK // 2  # 16 kv blocks per parity half
WARMUP_MMS = 3       # dummy matmuls to open the PE HAM clock gate at startup
# Schraudolph constants (fp16 bit pattern via int16 round/saturate):
# int16(round(1024/ln2 * u + 15*1024 - 44)) bitcast to fp16 ~= exp(u)
# (+-3% sawtooth, mean-centered so it mixes with the exact ACT stream).
A16 = float(1024.0 / math.log(2.0))
B16 = float(15 * 1024 - 44)

f32 = mybir.dt.float32
f16 = mybir.dt.float16
i16 = mybir.dt.int16
Exp = mybir.ActivationFunctionType.Exp
Mult, Add = mybir.AluOpType.mult, mybir.AluOpType.add

TILE_ORDER = [7, 6, 5, 4, 3, 2, 1, 0]

_prog_cache = {}


def _assignments():
    """[(pi, engine)] per tile, far->near then boundary. Full pairs alternate
    ACT/DVE ending with DVE just before the (always-ACT) boundary pair, so
    the two exp engines interleave tightly; 12 full pairs land on ACT, 16 on
    DVE -- balanced against their measured per-pair costs (1149 vs 1254 ns),
    with ACT also carrying the 8 boundary exps (~890 ns each)."""
    out = {}
    for T in range(NQT):
        pairs = []
        for pi in range(T):  # full pairs, far (low pi) first
            eng = "dve" if (T - 1 - pi) % 2 == 0 else "act"
            pairs.append((pi, eng))
        pairs.append((T, "act"))  # boundary (tile0's only pair)
        out[T] = pairs
    return out


def _build_program():
    if "nc" in _prog_cache:
        return _prog_cache["nc"]
    nc = bacc.Bacc("TRN2", target_bir_lowering=False, debug=False, num_devices=NCORES)

    qt_d = nc.dram_tensor("qt", [2 * D, S], f16, kind="ExternalInput").ap()
    kt_d = nc.dram_tensor("kt", [2 * D, PAR * BLK], f16, kind="ExternalInput").ap()
    vp_d = nc.dram_tensor("vp", [BLK, PAR * 65], f16, kind="ExternalInput").ap()
    mk_d = nc.dram_tensor("mk", [BLK, 2 * BLK], f16, kind="ExternalInput").ap()
    out_d = nc.dram_tensor("out", [65, S], f32, kind="ExternalOutput").ap()

    asg = _assignments()

    with tile.TileContext(nc) as tc, ExitStack() as ctx:
        const = ctx.enter_context(tc.tile_pool(name="const", bufs=1))
        ppool = ctx.enter_context(tc.tile_pool(name="pp", bufs=6))
        opool = ctx.enter_context(tc.tile_pool(name="op", bufs=3))
        sc_ps = ctx.enter_context(tc.tile_pool(name="scps", bufs=6, space="PSUM"))
        out_ps = ctx.enter_context(tc.tile_pool(name="ops", bufs=2, space="PSUM"))

        mk_s = const.tile([BLK, 2 * BLK], f16)
        kt_s = const.tile([2 * D, PAR * BLK], f16)
        vp_s = const.tile([BLK, PAR * 65], f16)
        qt_s = const.tile([2 * D, S], f16)
        # Input DMAs spread over three rings, first-use order (tile 7 first).
        nc.scalar.dma_start(kt_s[:, 0:256], kt_d[:, 0:256])
        nc.gpsimd.dma_start(vp_s[:], vp_d[:])
        nc.scalar.dma_start(kt_s[:, 256:1024], kt_d[:, 256:1024])
        nc.scalar.dma_start(kt_s[:, 1024:], kt_d[:, 1024:])
        nc.scalar.dma_start(mk_s[:], mk_d[:])
        for t in TILE_ORDER:
            nc.sync.dma_start(qt_s[:, t * QTW:(t + 1) * QTW], qt_d[:, t * QTW:(t + 1) * QTW])

        # PE warmup while input DMAs land (HAM clock gate -> 2.4 GHz).
        wsrc = const.tile([BLK, QTW], f16, name="wsrc")
        nc.vector.memset(wsrc[:], 0.0)
        wps = sc_ps.tile([BLK, QTW], f32, tag="sc", name="wps")
        for _ in range(WARMUP_MMS):
            nc.tensor.matmul(wps[:], wsrc[:, 0:BLK], wsrc[:], start=True, stop=True)

        # Global work list: one item per (tile, pair), in processing order.
        # Software pipeline with lag LAG: the PE emits the scores matmuls of
        # pair i+LAG before the exp/PV of pair i, so the in-order PE queue
        # never waits on the exp engines (the v2.0 per-pair emission made
        # every PV stall on its exp -- lockstep convoy, 58us).
        work = []
        for ti, T in enumerate(TILE_ORDER):
            pairs = asg[T]
            for n, (pi, eng) in enumerate(pairs):
                work.append({
                    "T": T, "pi": pi, "eng": eng,
                    "first": n == 0, "last": n == len(pairs) - 1,
                    "final_tile": ti == len(TILE_ORDER) - 1,
                })

        ops_by_tile = {}

        def emit_scores(it):
            T, pi = it["T"], it["pi"]
            # one single-bank PSUM tile per kv block: 6 rotating buffers give
            # the PE ~3 pairs of score run-ahead (2-bank pair tiles gave 1.5,
            # not enough elasticity to keep the PE queue from draining)
            sc_lo = sc_ps.tile([BLK, QTW], f32, tag="sc", name=f"sc{T}_{pi}l")
            sc_hi = sc_ps.tile([BLK, QTW], f32, tag="sc", name=f"sc{T}_{pi}h")
            it["sc"] = (sc_lo, sc_hi)
            if pi != T:
                for k, rg in ((0, 0), (1, D)):
                    blk = 2 * pi + k
                    nc.tensor.matmul(
                        (sc_lo if k == 0 else sc_hi)[:],
                        kt_s[rg:rg + D, blk * BLK:(blk + 1) * BLK],
                        qt_s[rg:rg + D, T * QTW:(T + 1) * QTW],
                        start=True, stop=True, tile_position=(rg, 0),
                    )
            else:
                # Boundary pair: parity blocks 2T (lo), 2T+1 (hi). sc layout:
                # [0:512] = lo scores (q cols 0:512), [512:768] = hi scores
                # (q cols 256:512). The per-core parity h is baked into the
                # DATA; the program uses the h=0 superset geometry and the
                # h-specific mask tensor zeroes the extra columns on h=1.
                lo, hi = 2 * T, 2 * T + 1
                nc.tensor.matmul(
                    sc_lo[:],
                    kt_s[0:D, lo * BLK:(lo + 1) * BLK],
                    qt_s[0:D, T * QTW:(T + 1) * QTW],
                    start=True, stop=True, tile_position=(0, 0),
                )
                nc.tensor.matmul(
                    sc_hi[:, 0:256],
                    kt_s[D:2 * D, hi * BLK:(hi + 1) * BLK],
                    qt_s[D:2 * D, T * QTW + 256:(T + 1) * QTW],
                    start=True, stop=True, tile_position=(D, 0),
                )

        def emit_finish(it):
            T, pi, eng = it["T"], it["pi"], it["eng"]
            sc_lo, sc_hi = it["sc"]
            if it["first"]:
                ops_by_tile[T] = [out_ps.tile([65, QTW], f32, tag="ops", name=f"ops{T}"), False]
            entry = ops_by_tile[T]
            ops = entry[0]

            def pv16(blkj, rhs_ap, col0, last):
                nc.tensor.matmul(
                    ops[:, col0:QTW],
                    vp_s[:, blkj * 65:(blkj + 1) * 65],
                    rhs_ap,
                    start=not entry[1],
                    stop=last,
                    skip_group_check=True,
                )
                entry[1] = True

            # exp is split into lo/hi block halves so the PV of a half can
            # start ~600ns after its scores instead of ~1.25us -- keeps the
            # PE queue from ever draining (PE idle gaps close the HAM clock
            # gate and halve the PE clock, which was the v2.1 death spiral).
            if pi != T:
                if eng == "act":
                    pt = ppool.tile([BLK, 2 * QTW], f16, tag="pt")
                    nc.scalar.activation(pt[:, 0:QTW], sc_lo[:], Exp, scale=0.125)
                    pv16(2 * pi, pt[:, 0:QTW], 0, False)
                    nc.scalar.activation(pt[:, QTW:2 * QTW], sc_hi[:], Exp, scale=0.125)
                    pv16(2 * pi + 1, pt[:, QTW:2 * QTW], 0, it["last"])
                else:
                    pq = ppool.tile([BLK, 2 * QTW], i16, tag="pt")
                    nc.vector.tensor_scalar(pq[:, 0:QTW], sc_lo[:], A16 * 0.125, B16, Mult, Add)
                    pv16(2 * pi, pq[:, 0:QTW].bitcast(f16), 0, False)
                    nc.vector.tensor_scalar(pq[:, QTW:2 * QTW], sc_hi[:], A16 * 0.125, B16, Mult, Add)
                    pv16(2 * pi + 1, pq[:, QTW:2 * QTW].bitcast(f16), 0, it["last"])
            else:
                lo, hi = 2 * T, 2 * T + 1
                pt = ppool.tile([BLK, 2 * QTW], f16, tag="pt")
                # boundary: one wider exp (the ~250ns per-instruction overhead
                # of a second half costs more than the latency it would hide
                # here -- boundary PVs sit mid-tile, absorbed by the LAG).
                # sc_lo/sc_hi are adjacent single-bank pool buffers only when
                # allocated back-to-back, so exp them separately but let the
                # lo one carry the full 512 and hi only its valid 256.
                nc.scalar.activation(pt[:, 0:QTW], sc_lo[:], Exp, scale=0.125)
                nc.scalar.activation(pt[:, QTW:QTW + 256], sc_hi[:, 0:256], Exp, scale=0.125)
                # diagonal sub-block masks (h-specific mk data) on gpsimd;
                # final tile's on DVE (gpsimd's ~670ns would sit in the tail)
                mask_eng = nc.vector if it["final_tile"] else nc.gpsimd
                mask_eng.tensor_mul(pt[:, 0:256], pt[:, 0:256], mk_s[:])
                pv16(lo, pt[:, 0:QTW], 0, False)
                mask_eng.tensor_mul(pt[:, QTW:QTW + 256], pt[:, QTW:QTW + 256], mk_s[:])
                pv16(hi, pt[:, QTW:QTW + 256], 256, it["last"])

            if it["last"]:
                osb = opool.tile([65, QTW], f32, tag="osb", name=f"osb{T}")
                if it["final_tile"]:
                    # copy+DMA in halves across both HWDGE rings so the
                    # exposed tail chain is half as long
                    nc.scalar.copy(osb[:, 0:256], ops[:, 0:256])
                    nc.scalar.dma_start(out_d[:, T * QTW:T * QTW + 256], osb[:, 0:256])
                    nc.vector.tensor_copy(osb[:, 256:], ops[:, 256:])
                    nc.sync.dma_start(out_d[:, T * QTW + 256:(T + 1) * QTW], osb[:, 256:])
                else:
                    # PSUM->SBUF copies: mostly DVE, two on ACT (rebalanced
                    # to the measured stream lengths; gpsimd cannot read PSUM)
                    if T in (6, 4):
                        nc.scalar.copy(osb[:], ops[:])
                    else:
                        nc.vector.tensor_copy(osb[:], ops[:])
                    nc.sync.dma_start(out_d[:, T * QTW:(T + 1) * QTW], osb[:])

        LAG = 4
        for i, it in enumerate(work):
            emit_scores(it)
            if i >= LAG:
                emit_finish(work[i - LAG])
        for it in work[-LAG:]:
            emit_finish(it)

    nc.compile()
    _prog_cache["nc"] = nc
    return nc


def kernel(query, key, value, padding):
    query = np.asarray(query, dtype=np.float32)
    key = np.asarray(key, dtype=np.float32)
    value = np.asarray(value, dtype=np.float32)
    padding = np.asarray(padding, dtype=bool)

    nc = _build_program()

    tri = (np.arange(BLK)[None, :] >= np.arange(BLK)[:, None]).astype(np.float16)

    in_maps = []
    for c in range(NCORES):
        b, h = divmod(c, 2)
        qt1 = np.ascontiguousarray(query[b].T).astype(np.float16)  # [64, 4096]
        qt = np.concatenate([qt1, qt1], axis=0)  # [128, 4096]
        kT = key[b].T  # [64, 4096] view
        blocks = [2 * i + h for i in range(PAR)]
        kt1 = np.ascontiguousarray(
            np.concatenate([kT[:, BLK * j:BLK * (j + 1)] for j in blocks], axis=1)
        ).astype(np.float16)
        kt = np.concatenate([kt1, kt1], axis=0)  # [128, 2048]

        vp = np.zeros((BLK, PAR * 65), dtype=np.float16)
        for i, j in enumerate(blocks):
            vblk = value[b, BLK * j:BLK * (j + 1), :].copy()
            pblk = padding[b, BLK * j:BLK * (j + 1)]
            vblk[pblk] = 0.0
            vp[:, 65 * i:65 * i + 64] = vblk
            vp[:, 65 * i + 64] = np.where(pblk, 0.0, 1.0)

        # Boundary mask, h-specific [128, 256], multiplied into pt cols
        # [0:256] (lo: q cols 0:256 of the tile) and [512:768] (hi: q cols
        # 256:512). In both cases the geometry is identical: h=0 -> diagonal
        # triangle in the first 128 cols, keep the next 128; h=1 -> first
        # 128 cols fully masked, triangle in the next 128.
        mk = np.zeros((BLK, 2 * BLK), dtype=np.float16)
        if h == 0:
            mk[:, 0:128] = tri
            mk[:, 128:256] = 1.0
        else:
            mk[:, 128:256] = tri
        in_maps.append({"qt": qt, "kt": kt, "vp": vp, "mk": mk})

    global _last_in_maps
    _last_in_maps = in_maps
    res = run_bass_kernel_spmd(nc, in_maps, list(range(NCORES)))

    out = np.empty((B, S, D), dtype=np.float32)
    for b in range(B):
        r0 = res.results[2 * b]["out"].astype(np.float64)
        r1 = res.results[2 * b + 1]["out"].astype(np.float64)
        num = r0[:64] + r1[:64]
        den = r0[64] + r1[64]
        out[b] = (num / den).T.astype(np.float32)
    return out
